# revision 1
# baseline (speedup 1.0000x reference)
"""Trainium2 Bass kernel for nn_Attention_20976620274235 (sparse attention).

Sharding: 8 cores = 4 batches x 2 head-groups (8 heads each).
Per-core SPMD program:
  phase 1: QKV projection (fp16 matmuls, fp32 PSUM accum) + RoPE (fp32r)
           q/k produced head-dim-major [hd, s]; V produced s-major fp16.
  phase 2: attention with TRANSPOSED scores sT[k, q] = kT.T @ qT so the
           softmax column-mask is a per-partition bias on the Exp activation
           and P^T feeds P@V directly as the stationary operand (no transposes).
           Unnormalized accumulate; divide by (ones^T @ P^T) at the end.
  phase 3: output projection (fp16), partial over this core's 1024 channels.
Host: sums the two head-group partials per batch.
"""

import math

import numpy as np

import concourse.bass as bass
from concourse import bacc
import concourse.mybir as mybir
import concourse.tile as tile
from concourse.bass_utils import run_bass_kernel_spmd

B, S, DIM, H = 4, 2048, 2048, 16
HD = 128          # head dim
NC = 8            # cores
HC = 8            # heads per core
CC = HC * HD      # 1024 channels per core
F32 = mybir.dt.float32
F32R = mybir.dt.float32r
F16 = mybir.dt.float16
SM_SCALE = 1.0 / math.sqrt(HD)
EXP_BIAS = -14.0  # safe shift; softmax ratio is shift-invariant
MASK_BIAS = -1.0e9

_CACHE = {}


def _build_program():
    nc = bacc.Bacc("TRN2", target_bir_lowering=False, debug=False, num_devices=NC)

    xT = nc.dram_tensor("xT", [DIM, S], F16, kind="ExternalInput").ap()
    wqkT = nc.dram_tensor("wqkT", [DIM, 2 * CC], F16, kind="ExternalInput").ap()
    wvT = nc.dram_tensor("wvT", [DIM, CC], F16, kind="ExternalInput").ap()
    woT = nc.dram_tensor("woT", [CC, DIM], F16, kind="ExternalInput").ap()
    cosT = nc.dram_tensor("cosT", [HD, S], F32, kind="ExternalInput").ap()
    sinT = nc.dram_tensor("sinT", [HD, S], F32, kind="ExternalInput").ap()
    cbias = nc.dram_tensor("cbias", [128, 16], F32, kind="ExternalInput").ap()
    rmask = nc.dram_tensor("rmask", [1, S], F32, kind="ExternalInput").ap()
    rotT = nc.dram_tensor("rotT", [HD, HD], F16, kind="ExternalInput").ap()
    qkT = nc.dram_tensor("qkT", [2 * CC, S], F16).ap()  # internal scratch
    out = nc.dram_tensor("out", [S, DIM], F32, kind="ExternalOutput").ap()

    Exp = mybir.ActivationFunctionType.Exp

    with tile.TileContext(nc) as tc:
        with tc.tile_pool(name="consts", bufs=1) as cpool, \
             tc.tile_pool(name="vall", bufs=1) as vpool, \
             tc.tile_pool(name="otall", bufs=1) as opool:
            cos_sb = cpool.tile([HD, S], F32)
            sin_sb = cpool.tile([HD, S], F32)
            cb_sb = cpool.tile([128, 16], F32)
            rm_sb = cpool.tile([1, S], F32)
            rt_sb = cpool.tile([HD, HD], F16)
            ones_sb = cpool.tile([128, 1], F16)
            onesr_sb = cpool.tile([1, 128], F16)
            nc.gpsimd.memset(onesr_sb[:], 1.0)
            nc.sync.dma_start(cos_sb[:], cosT[:])
            nc.sync.dma_start(sin_sb[:], sinT[:])
            nc.sync.dma_start(cb_sb[:], cbias[:])
            nc.sync.dma_start(rm_sb[:], rmask[:])
            nc.sync.dma_start(rt_sb[:], rotT[:])
            nc.gpsimd.memset(ones_sb[:], 1.0)

            V_all = vpool.tile([128, 16 * CC], F16)  # [s%128, t*1024 + ch]
            OT_all = opool.tile([128, HC * S], F16)  # [hd, h*2048 + s]

            # ---------------- phase 1: QKV + RoPE ----------------
            with tc.tile_pool(name="xn", bufs=1) as xpool, \
                 tc.tile_pool(name="wv", bufs=1) as wvpool, \
                 tc.tile_pool(name="wm", bufs=2) as wmpool, \
                 tc.tile_pool(name="rope", bufs=2) as rpool, \
                 tc.tile_pool(name="psqk", bufs=2, space="PSUM") as psqk, \
                 tc.tile_pool(name="psrot", bufs=2, space="PSUM") as psrot, \
                 tc.tile_pool(name="psv", bufs=2, space="PSUM") as psv:
                wv_sb = wvpool.tile([128, 16 * CC], F16)
                nc.sync.dma_start(
                    wv_sb[:].rearrange("p (t c) -> p t c", t=16),
                    wvT[:].rearrange("(t p) c -> p t c", p=128),
                )
                for n in range(4):  # s-chunks of 512
                    ns = slice(n * 512, (n + 1) * 512)
                    xn = xpool.tile([128, 16 * 512], F16, tag="xn")
                    nc.sync.dma_start(
                        xn[:].rearrange("p (t s) -> p t s", t=16),
                        xT[:, ns].rearrange("(t p) s -> p t s", p=128),
                    )
                    # q/k channel tiles (m 0..7 = q heads, 8..15 = k heads)
                    for m in range(16):
                        wm = wmpool.tile([128, 16 * 128], F16, tag="wm")
                        nc.sync.dma_start(
                            wm[:].rearrange("p (t c) -> p t c", t=16),
                            wqkT[:, m * 128:(m + 1) * 128].rearrange(
                                "(t p) c -> p t c", p=128
                            ),
                        )
                        ps = psqk.tile([128, 512], F32, tag="psqk")
                        for t in range(16):
                            nc.tensor.matmul(
                                ps[:],
                                lhsT=wm[:, t * 128:(t + 1) * 128],
                                rhs=xn[:, t * 512:(t + 1) * 512],
                                start=(t == 0),
                                stop=(t == 15),
                            )
                        qraw = rpool.tile([128, 512], F16, tag="qraw")
                        nc.scalar.copy(qraw[:], ps[:])
                        pr = psrot.tile([128, 512], F32, tag="psrot")
                        nc.tensor.matmul(
                            pr[:],
                            lhsT=rt_sb[:],
                            rhs=qraw[:],
                            start=True,
                            stop=True,
                        )
                        t1 = rpool.tile([128, 512], F32, tag="t1")
                        nc.vector.tensor_mul(t1[:], qraw[:], cos_sb[:, ns])
                        t2 = rpool.tile([128, 512], F32, tag="t2")
                        nc.vector.tensor_mul(t2[:], pr[:], sin_sb[:, ns])
                        qf = rpool.tile([128, 512], F16, tag="qf")
                        nc.vector.tensor_add(qf[:], t1[:], t2[:])
                        nc.sync.dma_start(qkT[m * 128:(m + 1) * 128, ns], qf[:])
                    # V tiles: out [s-tile, 1024] s-major
                    for j in range(4):
                        jj = n * 4 + j
                        for half in range(2):
                            pv = psv.tile([128, 512], F32, tag="psv")
                            for t in range(16):
                                nc.tensor.matmul(
                                    pv[:],
                                    lhsT=xn[:, t * 512 + j * 128: t * 512 + (j + 1) * 128],
                                    rhs=wv_sb[:, t * CC + half * 512: t * CC + (half + 1) * 512],
                                    start=(t == 0),
                                    stop=(t == 15),
                                )
                            for hh in range(4):
                                h = half * 4 + hh
                                nc.vector.tensor_copy(
                                    V_all[:, jj * CC + h * 128: jj * CC + (h + 1) * 128],
                                    pv[:, hh * 128:(hh + 1) * 128],
                                )

            # ---------------- phase 2: attention ----------------
            with tc.tile_pool(name="kq", bufs=2) as kqpool, \
                 tc.tile_pool(name="pt", bufs=3) as ptpool, \
                 tc.tile_pool(name="sml", bufs=2) as smpool, \
                 tc.tile_pool(name="pss", bufs=2, space="PSUM") as pss, \
                 tc.tile_pool(name="pso", bufs=2, space="PSUM") as pso, \
                 tc.tile_pool(name="psd", bufs=2, space="PSUM") as psd, \
                 tc.tile_pool(name="psb", bufs=2, space="PSUM") as psb:
                for h in range(HC):
                    kT = kqpool.tile([128, S], F16, tag="kT")
                    qT = kqpool.tile([128, S], F16, tag="qT")
                    nc.sync.dma_start(kT[:], qkT[CC + h * 128: CC + (h + 1) * 128, :])
                    nc.sync.dma_start(qT[:], qkT[h * 128:(h + 1) * 128, :])
                    for q4 in range(4):
                        qs = slice(q4 * 512, (q4 + 1) * 512)
                        po = pso.tile([128, 512], F32, tag="pso")
                        pd = psd.tile([1, 512], F32, tag="psd")
                        for t in range(16):
                            sps = pss.tile([128, 512], F32, tag="pss")
                            nc.tensor.matmul(
                                sps[:],
                                lhsT=kT[:, t * 128:(t + 1) * 128],
                                rhs=qT[:, qs],
                                start=True,
                                stop=True,
                            )
                            pt = ptpool.tile([128, 512], F16, tag="pt")
                            nc.scalar.activation(
                                pt[:], sps[:], Exp,
                                bias=cb_sb[:, t:t + 1], scale=SM_SCALE,
                            )
                            nc.tensor.matmul(
                                po[:],
                                lhsT=V_all[:, t * CC + h * 128: t * CC + (h + 1) * 128],
                                rhs=pt[:],
                                start=(t == 0),
                                stop=(t == 15),
                            )
                            nc.tensor.matmul(
                                pd[:],
                                lhsT=ones_sb[:],
                                rhs=pt[:],
                                start=(t == 0),
                                stop=(t == 15),
                            )
                        den = smpool.tile([1, 512], F32, tag="den")
                        nc.scalar.copy(den[:], pd[:])
                        rec = smpool.tile([1, 512], F32, tag="rec")
                        nc.vector.reciprocal(rec[:], den[:])
                        rmc = smpool.tile([1, 512], F16, tag="rmc")
                        nc.vector.tensor_mul(rmc[:], rec[:], rm_sb[:, qs])
                        # broadcast rmc across partitions via K=1 matmul
                        bcp = psb.tile([128, 512], F32, tag="psb")
                        nc.tensor.matmul(
                            bcp[:],
                            lhsT=onesr_sb[:],
                            rhs=rmc[:],
                            start=True,
                            stop=True,
                        )
                        bcs = smpool.tile([128, 512], F32, tag="bcs")
                        nc.scalar.copy(bcs[:], bcp[:])
                        nc.vector.tensor_mul(
                            OT_all[:, h * S + q4 * 512: h * S + (q4 + 1) * 512],
                            po[:],
                            bcs[:],
                        )

            # ---------------- phase 3: output projection ----------------
            with tc.tile_pool(name="wo", bufs=1) as wopool, \
                 tc.tile_pool(name="ob", bufs=2) as obpool, \
                 tc.tile_pool(name="psf", bufs=2, space="PSUM") as psf:
                wo_sb = wopool.tile([128, 8 * DIM], F16)
                nc.sync.dma_start(
                    wo_sb[:].rearrange("p (d o) -> p d o", d=8),
                    woT[:].rearrange("(d p) o -> p d o", p=128),
                )
                for sj in range(16):
                    for oc in range(4):
                        pf = psf.tile([128, 512], F32, tag="psf")
                        for d in range(8):
                            nc.tensor.matmul(
                                pf[:],
                                lhsT=OT_all[:, d * S + sj * 128: d * S + (sj + 1) * 128],
                                rhs=wo_sb[:, d * DIM + oc * 512: d * DIM + (oc + 1) * 512],
                                start=(d == 0),
                                stop=(d == 7),
                            )
                        ob = obpool.tile([128, 512], F32, tag="ob")
                        nc.scalar.copy(ob[:], pf[:])
                        nc.sync.dma_start(
                            out[sj * 128:(sj + 1) * 128, oc * 512:(oc + 1) * 512],
                            ob[:],
                        )
    nc.compile()
    return nc


def _host_shards(x, freqs_cos, freqs_sin, vis_mask, wqkv, wo):
    cosT = np.ascontiguousarray(freqs_cos[0, :, 0, :].T, dtype=np.float32)
    sinT = np.ascontiguousarray(freqs_sin[0, :, 0, :].T, dtype=np.float32)
    rotT = np.zeros((HD, HD), dtype=np.float16)
    for i in range(HD // 2):
        rotT[2 * i + 1, 2 * i] = -1.0
        rotT[2 * i, 2 * i + 1] = 1.0
    in_maps = []
    for c in range(NC):
        b, g = c // 2, c % 2
        vis = vis_mask[b].astype(np.float32)
        xb = (x[b] * vis[:, None]).astype(np.float32)
        xT = np.ascontiguousarray(xb.T).astype(np.float16)
        wq = wqkv[g * CC:(g + 1) * CC]
        wk = wqkv[DIM + g * CC: DIM + (g + 1) * CC]
        wv = wqkv[2 * DIM + g * CC: 2 * DIM + (g + 1) * CC]
        wqkT = np.ascontiguousarray(
            np.concatenate([wq, wk], axis=0).T
        ).astype(np.float16)
        wvT = np.ascontiguousarray(wv.T).astype(np.float16)
        woT = np.ascontiguousarray(wo[:, g * CC:(g + 1) * CC].T).astype(np.float16)
        cbias = np.where(vis > 0, EXP_BIAS, MASK_BIAS).astype(np.float32)
        cbias = np.ascontiguousarray(cbias.reshape(16, 128).T)  # [p, t]
        rmask = vis.reshape(1, S).astype(np.float32)
        in_maps.append({
            "xT": xT, "wqkT": wqkT, "wvT": wvT, "woT": woT,
            "cosT": cosT, "sinT": sinT, "cbias": cbias,
            "rmask": rmask, "rotT": rotT,
        })
    return in_maps


def kernel(x, freqs_cos, freqs_sin, vis_mask, wqkv, wo):
    x = np.asarray(x, dtype=np.float32)
    freqs_cos = np.asarray(freqs_cos, dtype=np.float32)
    freqs_sin = np.asarray(freqs_sin, dtype=np.float32)
    vis_mask = np.asarray(vis_mask)
    wqkv = np.asarray(wqkv, dtype=np.float32)
    wo = np.asarray(wo, dtype=np.float32)

    if "nc" not in _CACHE:
        _CACHE["nc"] = _build_program()
    nc = _CACHE["nc"]
    in_maps = _host_shards(x, freqs_cos, freqs_sin, vis_mask, wqkv, wo)
    res = run_bass_kernel_spmd(nc, in_maps, core_ids=list(range(NC)))
    outs = [r["out"] for r in res.results]
    final = np.empty((B, S, DIM), dtype=np.float32)
    for b in range(B):
        final[b] = outs[2 * b] + outs[2 * b + 1]
    return final



# revision 7
# speedup vs baseline: 2.5484x; 2.5484x over previous
"""Trainium2 Bass kernel for nn_Attention_20976620274235 (sparse attention).

Key idea: vis_mask rows/cols that are masked out contribute exactly zero to
the output (masked q rows give attn=0 -> out row 0; masked k positions are
excluded from the softmax).  So we COMPACT: host gathers the ~S/2 visible
positions per batch, pads to SPAD=1152 (=9*128, +5.7 sigma above the
Binomial(2048,.5) mean), the device computes attention on the short
sequence, and the host scatters rows back (zeros elsewhere).

Sharding: 8 cores = 4 batches x 2 head-groups (8 heads each).
Per-core SPMD program (all fp16 matmuls, fp32 PSUM):
  phase 1: QKV projection + RoPE; q/k kept SBUF-resident head-dim-major
           [hd, s]; V s-major fp16.  RoPE chain is software-pipelined one
           chunk behind the projection matmuls.
  phase 2: attention with TRANSPOSED scores sT[k, q] = kT.T @ qT so the
           padded-column mask is a per-partition bias on the Exp activation
           and P^T feeds P@V directly as the moving operand.  Scores run
           two k-tiles ahead of PV/denominator so the scalar-engine Exp
           latency stays off the PE critical path.  Unnormalized
           accumulate; divide by (ones^T @ P^T) at the end.
  phase 3: output projection, partial over this core's 1024 channels.
Host: sums the two head-group partials per batch, scatters visible rows.

Engine assignment: PE matmuls; scalar engine does ONLY Exp; Pool (gpsimd)
does PSUM->SBUF copies; DVE does RoPE muls/adds, reciprocal, final scale.
"""

import math

import numpy as np

import concourse.bass as bass
from concourse import bacc
import concourse.mybir as mybir
import concourse.tile as tile
from concourse.bass_utils import run_bass_kernel_spmd

B, S, DIM, H = 4, 2048, 2048, 16
HD = 128          # head dim
NC = 8            # cores
HC = 8            # heads per core
CC = HC * HD      # 1024 channels per core
SPAD = 1152       # padded compacted sequence length (9 * 128)
F32 = mybir.dt.float32
F16 = mybir.dt.float16
SM_SCALE = 1.0 / math.sqrt(HD)
EXP_BIAS = -6.0   # shift-invariant; keeps exp() in f16 normal range
MASK_BIAS = -1.0e9

_CACHE = {}


def _build_program(spad):
    nt = spad // 128  # number of 128-wide position tiles
    chunks = []
    off = 0
    while off < spad:
        cw = min(512, spad - off)
        chunks.append((off, cw))
        off += cw

    nc = bacc.Bacc("TRN2", target_bir_lowering=False, debug=False, num_devices=NC)

    # host-pretiled inputs: layouts match SBUF exactly (contiguous DMAs)
    xg = nc.dram_tensor("xg", [128, 16 * spad], F16, kind="ExternalInput").ap()
    wqk = nc.dram_tensor("wqk", [128, 16 * 16 * 128], F16, kind="ExternalInput").ap()
    wv = nc.dram_tensor("wv", [128, 16 * CC], F16, kind="ExternalInput").ap()
    wo = nc.dram_tensor("wo", [128, 8 * DIM], F16, kind="ExternalInput").ap()
    cosg = nc.dram_tensor("cosg", [HD, spad], F16, kind="ExternalInput").ap()
    sing = nc.dram_tensor("sing", [HD, spad], F16, kind="ExternalInput").ap()
    cbias = nc.dram_tensor("cbias", [128, nt], F32, kind="ExternalInput").ap()
    rotT = nc.dram_tensor("rotT", [HD, HD], F16, kind="ExternalInput").ap()
    out = nc.dram_tensor("out", [spad, DIM], F16, kind="ExternalOutput").ap()

    Exp = mybir.ActivationFunctionType.Exp

    with tile.TileContext(nc) as tc:
        with tc.tile_pool(name="consts", bufs=1) as cpool, \
             tc.tile_pool(name="persist", bufs=1) as ppool, \
             tc.tile_pool(name="wop", bufs=1) as wopool:
            cos_sb = cpool.tile([HD, spad], F16)
            sin_sb = cpool.tile([HD, spad], F16)
            cb_sb = cpool.tile([128, nt], F32)
            rt_sb = cpool.tile([HD, HD], F16)
            ones_sb = cpool.tile([128, 1], F16)
            onesr_sb = cpool.tile([1, 128], F16)
            nc.gpsimd.memset(ones_sb[:], 1.0)
            nc.gpsimd.memset(onesr_sb[:], 1.0)
            nc.sync.dma_start(cos_sb[:], cosg[:])
            nc.sync.dma_start(sin_sb[:], sing[:])
            nc.sync.dma_start(cb_sb[:], cbias[:])
            nc.sync.dma_start(rt_sb[:], rotT[:])

            qk_all = ppool.tile([128, 16 * spad], F16)  # [hd, m*spad + pos]
            V_all = ppool.tile([128, nt * CC], F16)     # [s%128, j*CC + ch]
            OT_all = ppool.tile([128, HC * spad], F16)  # [hd, h*spad + pos]
            wo_sb = wopool.tile([128, 8 * DIM], F16)

            # ---------------- phase 1: QKV + RoPE ----------------
            with tc.tile_pool(name="xp", bufs=1) as xpool, \
                 tc.tile_pool(name="wvp", bufs=1) as wvpool, \
                 tc.tile_pool(name="wmp", bufs=2) as wmpool, \
                 tc.tile_pool(name="rp", bufs=2) as rpool, \
                 tc.tile_pool(name="psqk", bufs=2, space="PSUM") as psqk, \
                 tc.tile_pool(name="psrot", bufs=2, space="PSUM") as psrot, \
                 tc.tile_pool(name="psv", bufs=2, space="PSUM") as psv:
                x_sb = xpool.tile([128, 16 * spad], F16)
                nc.sync.dma_start(x_sb[:], xg[:])
                wv_sb = wvpool.tile([128, 16 * CC], F16)
                nc.sync.dma_start(wv_sb[:], wv[:])

                # RoPE chain runs one (m, chunk) behind the projection
                # matmuls so the PE never waits on the Pool-engine copy.
                pending = [None]

                def flush_rope():
                    if pending[0] is None:
                        return
                    ps, m, c0, cw = pending[0]
                    pending[0] = None
                    qraw = rpool.tile([128, 512], F16, tag="qraw")
                    nc.scalar.copy(qraw[:, :cw], ps[:, :cw])
                    pr = psrot.tile([128, 512], F32, tag="psrot")
                    nc.tensor.matmul(pr[:, :cw], lhsT=rt_sb[:], rhs=qraw[:, :cw],
                                     start=True, stop=True)
                    t1 = rpool.tile([128, 512], F16, tag="t1")
                    nc.vector.tensor_mul(t1[:, :cw], qraw[:, :cw],
                                         cos_sb[:, c0:c0 + cw])
                    t2 = rpool.tile([128, 512], F16, tag="t2")
                    nc.vector.tensor_mul(t2[:, :cw], pr[:, :cw],
                                         sin_sb[:, c0:c0 + cw])
                    nc.vector.tensor_add(
                        qk_all[:, m * spad + c0: m * spad + c0 + cw],
                        t1[:, :cw], t2[:, :cw])

                for m in range(16):  # m 0..7 q heads, 8..15 k heads
                    wm = wmpool.tile([128, 16 * 128], F16, tag="wm")
                    nc.sync.dma_start(wm[:], wqk[:, m * 2048:(m + 1) * 2048])
                    for c0, cw in chunks:
                        ps = psqk.tile([128, 512], F32, tag="psqk")
                        for t in range(16):
                            nc.tensor.matmul(
                                ps[:, :cw],
                                lhsT=wm[:, t * 128:(t + 1) * 128],
                                rhs=x_sb[:, t * spad + c0: t * spad + c0 + cw],
                                start=(t == 0), stop=(t == 15))
                        flush_rope()
                        pending[0] = (ps, m, c0, cw)
                flush_rope()

                for j in range(nt):  # V: out [pos, vch] s-major
                    for half in range(2):
                        pv = psv.tile([128, 512], F32, tag="psv")
                        for t in range(16):
                            nc.tensor.matmul(
                                pv[:],
                                lhsT=x_sb[:, t * spad + j * 128: t * spad + (j + 1) * 128],
                                rhs=wv_sb[:, t * CC + half * 512: t * CC + (half + 1) * 512],
                                start=(t == 0), stop=(t == 15))
                        nc.scalar.copy(
                            V_all[:, j * CC + half * 512: j * CC + (half + 1) * 512],
                            pv[:])

            # ---------------- phase 2: attention ----------------
            with tc.tile_pool(name="ptp", bufs=4) as ptpool, \
                 tc.tile_pool(name="smp", bufs=2) as smpool, \
                 tc.tile_pool(name="pss", bufs=3, space="PSUM") as pss, \
                 tc.tile_pool(name="pso", bufs=2, space="PSUM") as pso, \
                 tc.tile_pool(name="psd", bufs=2, space="PSUM") as psd, \
                 tc.tile_pool(name="psb", bufs=1, space="PSUM") as psb:
                nc.sync.dma_start(wo_sb[:], wo[:])  # prefetch for phase 3
                for h in range(HC):
                    kbase = (8 + h) * spad
                    qbase = h * spad
                    for c0, cw in chunks:
                        po = pso.tile([128, 512], F32, tag="po")
                        pd = psd.tile([1, 512], F32, tag="pd")
                        pt_l = {}
                        # scores run 2 k-tiles ahead of PV/denominator
                        for t in range(nt + 2):
                            if t < nt:
                                sp = pss.tile([128, 512], F32, tag="sps")
                                nc.tensor.matmul(
                                    sp[:, :cw],
                                    lhsT=qk_all[:, kbase + t * 128: kbase + (t + 1) * 128],
                                    rhs=qk_all[:, qbase + c0: qbase + c0 + cw],
                                    start=True, stop=True)
                                pt = ptpool.tile([128, 512], F16, tag="pt")
                                nc.scalar.activation(
                                    pt[:, :cw], sp[:, :cw], Exp,
                                    bias=cb_sb[:, t:t + 1], scale=SM_SCALE)
                                pt_l[t] = pt
                            tt = t - 2
                            if 0 <= tt < nt:
                                pt = pt_l.pop(tt)
                                nc.tensor.matmul(
                                    po[:, :cw],
                                    lhsT=V_all[:, tt * CC + h * 128: tt * CC + (h + 1) * 128],
                                    rhs=pt[:, :cw],
                                    start=(tt == 0), stop=(tt == nt - 1))
                                nc.tensor.matmul(
                                    pd[:, :cw],
                                    lhsT=ones_sb[:],
                                    rhs=pt[:, :cw],
                                    start=(tt == 0), stop=(tt == nt - 1))
                        den = smpool.tile([1, 512], F32, tag="den")
                        nc.vector.tensor_copy(den[:, :cw], pd[:, :cw])
                        rec = smpool.tile([1, 512], F32, tag="rec")
                        nc.vector.reciprocal_approx_fast(rec[:, :cw], den[:, :cw])
                        rec16 = smpool.tile([1, 512], F16, tag="rec16")
                        nc.vector.tensor_copy(rec16[:, :cw], rec[:, :cw])
                        # broadcast 1/den across partitions via K=1 matmul
                        bcp = psb.tile([128, 512], F32, tag="bcp")
                        nc.tensor.matmul(bcp[:, :cw], lhsT=onesr_sb[:],
                                         rhs=rec16[:, :cw], start=True, stop=True)
                        bcs = smpool.tile([128, 512], F16, tag="bcs")
                        nc.vector.tensor_copy(bcs[:, :cw], bcp[:, :cw])
                        nc.vector.tensor_mul(
                            OT_all[:, h * spad + c0: h * spad + c0 + cw],
                            po[:, :cw], bcs[:, :cw])

            # ---------------- phase 3: output projection ----------------
            with tc.tile_pool(name="obp", bufs=2) as obpool, \
                 tc.tile_pool(name="psf", bufs=2, space="PSUM") as psf:
                for sj in range(nt):
                    for oc in range(4):
                        pf = psf.tile([128, 512], F32, tag="pf")
                        for hh in range(8):
                            nc.tensor.matmul(
                                pf[:],
                                lhsT=OT_all[:, hh * spad + sj * 128: hh * spad + (sj + 1) * 128],
                                rhs=wo_sb[:, hh * DIM + oc * 512: hh * DIM + (oc + 1) * 512],
                                start=(hh == 0), stop=(hh == 7))
                        ob = obpool.tile([128, 512], F16, tag="ob")
                        nc.scalar.copy(ob[:], pf[:])
                        nc.sync.dma_start(
                            out[sj * 128:(sj + 1) * 128, oc * 512:(oc + 1) * 512],
                            ob[:])
    nc.compile()
    return nc


def _rot_matrix():
    rotT = np.zeros((HD, HD), dtype=np.float16)
    for i in range(HD // 2):
        rotT[2 * i + 1, 2 * i] = -1.0
        rotT[2 * i, 2 * i + 1] = 1.0
    return rotT


def _host_shards(x, freqs_cos, freqs_sin, vis_mask, wqkv, wo, spad=SPAD):
    x = np.asarray(x, dtype=np.float32)
    freqs_cos = np.asarray(freqs_cos, dtype=np.float32)
    freqs_sin = np.asarray(freqs_sin, dtype=np.float32)
    vis = np.asarray(vis_mask).astype(bool)
    wqkv = np.asarray(wqkv, dtype=np.float32)
    wo = np.asarray(wo, dtype=np.float32)
    nt = spad // 128
    rotT = _rot_matrix()

    # per-head-group weights (shared by cores with the same g)
    wmats = []
    for g in range(2):
        wq = wqkv[g * CC:(g + 1) * CC]
        wk = wqkv[DIM + g * CC: DIM + (g + 1) * CC]
        wqk_full = np.concatenate([wq, wk], axis=0)  # [2048 ch, 2048 dim]
        wqk_t = np.ascontiguousarray(
            wqk_full.T.reshape(16, 128, 16, 128).transpose(1, 2, 0, 3)
            .reshape(128, 16 * 16 * 128)).astype(np.float16)
        wv_g = wqkv[2 * DIM + g * CC: 2 * DIM + (g + 1) * CC]  # [1024, 2048]
        wv_t = np.ascontiguousarray(
            wv_g.T.reshape(16, 128, CC).transpose(1, 0, 2)
            .reshape(128, 16 * CC)).astype(np.float16)
        wo_g = wo[:, g * CC:(g + 1) * CC]  # [2048 out, 1024 d]
        wo_t = np.ascontiguousarray(
            wo_g.T.reshape(8, 128, DIM).transpose(1, 0, 2)
            .reshape(128, 8 * DIM)).astype(np.float16)
        wmats.append((wqk_t, wv_t, wo_t))

    # per-batch gathered tensors (shared by cores with the same b)
    bmats = []
    for b in range(B):
        idx = np.nonzero(vis[b])[0]
        sv = len(idx)
        assert sv <= spad
        xp = np.zeros((spad, DIM), dtype=np.float32)
        xp[:sv] = x[b][idx]
        xg = np.ascontiguousarray(
            xp.T.reshape(16, 128, spad).transpose(1, 0, 2)
            .reshape(128, 16 * spad)).astype(np.float16)
        cp = np.zeros((spad, HD), dtype=np.float32)
        cp[:sv] = freqs_cos[0, idx, 0, :]
        sp = np.zeros((spad, HD), dtype=np.float32)
        sp[:sv] = freqs_sin[0, idx, 0, :]
        cosg = np.ascontiguousarray(cp.T).astype(np.float16)
        sing = np.ascontiguousarray(sp.T).astype(np.float16)
        valid = np.arange(spad) < sv
        cb = np.where(valid, EXP_BIAS, MASK_BIAS).astype(np.float32)
        cbias = np.ascontiguousarray(cb.reshape(nt, 128).T)  # [p, t]
        bmats.append((xg, cosg, sing, cbias))

    in_maps = []
    for c in range(NC):
        b, g = c // 2, c % 2
        wqk_t, wv_t, wo_t = wmats[g]
        xg, cosg, sing, cbias = bmats[b]
        in_maps.append({
            "xg": xg, "wqk": wqk_t, "wv": wv_t, "wo": wo_t,
            "cosg": cosg, "sing": sing, "cbias": cbias, "rotT": rotT,
        })
    return in_maps


def _numpy_fallback(x, freqs_cos, freqs_sin, vis_mask, wqkv, wo):
    # exact reference math; only used if a batch has > SPAD visible rows
    # (impossible for Bernoulli(0.5) masks, kept for safety)
    x = np.asarray(x, dtype=np.float32)
    fc = np.asarray(freqs_cos, dtype=np.float32)
    fs = np.asarray(freqs_sin, dtype=np.float32)
    vis = np.asarray(vis_mask).astype(bool)
    wqkv = np.asarray(wqkv, dtype=np.float32)
    wo = np.asarray(wo, dtype=np.float32)
    qkv = np.einsum('bsd,od->bso', x, wqkv)
    xq, xk, xv = np.split(qkv, 3, axis=-1)
    xq = xq.reshape(B, S, H, HD)
    xk = xk.reshape(B, S, H, HD)
    xv = xv.reshape(B, S, H, HD)

    def rot(t):
        t2 = t.reshape(t.shape[:-1] + (-1, 2))
        r = np.stack([-t2[..., 1], t2[..., 0]], axis=-1)
        return r.reshape(t.shape)

    xq = xq * fc + rot(xq) * fs
    xk = xk * fc + rot(xk) * fs
    s = np.einsum('bqhd,bkhd->bhqk', xq, xk) * SM_SCALE
    am = (vis[:, None, :, None] & vis[:, None, None, :])
    s = np.where(am, s, -np.inf)
    m = np.maximum(np.max(s, axis=-1, keepdims=True), np.float32(-1e20))
    p = np.where(am, np.exp(s - m), 0.0)
    denom = np.maximum(np.sum(p, axis=-1, keepdims=True), np.float32(1e-6))
    attn = p / denom
    o = np.einsum('bhqk,bkhd->bqhd', attn, xv).reshape(B, S, DIM)
    return np.einsum('bsd,od->bso', o, wo).astype(np.float32)


def kernel(x, freqs_cos, freqs_sin, vis_mask, wqkv, wo):
    vis = np.asarray(vis_mask).astype(bool)
    svs = [int(vis[b].sum()) for b in range(B)]
    if max(svs) > SPAD:
        return _numpy_fallback(x, freqs_cos, freqs_sin, vis_mask, wqkv, wo)

    if "nc" not in _CACHE:
        _CACHE["nc"] = _build_program(SPAD)
    nc = _CACHE["nc"]
    in_maps = _host_shards(x, freqs_cos, freqs_sin, vis_mask, wqkv, wo)
    res = run_bass_kernel_spmd(nc, in_maps, core_ids=list(range(NC)))
    outs = [r["out"] for r in res.results]  # [SPAD, DIM] f16 partials
    final = np.zeros((B, S, DIM), dtype=np.float32)
    for b in range(B):
        idx = np.nonzero(vis[b])[0]
        sv = len(idx)
        final[b][idx] = (outs[2 * b][:sv].astype(np.float32)
                         + outs[2 * b + 1][:sv].astype(np.float32))
    return final
